# revision 44
# baseline (speedup 1.0000x reference)
"""Trainium2 Bass kernel for nn_CausalSelfAttention_8237747274097.

Reference math (single-head attention over full n_embd=1024, scale 1/8):
    qkv = x @ W_attn + b_attn ; q,k,v = split(qkv)
    att = softmax(causal(q @ k.T / 8)) ; y = att @ v ; out = y @ W_proj + b_proj

Sharding (8 cores): core c = (batch b = c//2, parity p = c%2). Each core owns 8
of the 16 query row-tiles (128 rows each) of its batch, interleaved/paired so
causal work is balanced, and computes full K/V for the batch. Outputs are
disjoint row slices -> host gather is a pure scatter + bias add.

Math simplifications (all exact):
  - k bias drops out of softmax; v bias folds into b_eff = b_proj + b_v@W_proj.
  - 1/8 q scale folded into W_q/b_q; V scaled 1/8 and W_proj scaled 8 (exact
    pow-2) so fp16-stored attention numerators stay far from fp16 overflow.
Softmax is computed without max-subtraction (scores are O(3); exp is safe) so
the denominator comes free from a ones-row matmul.

Precision: all matmul operands fp16 (full PE rate, FWL-eligible weight loads,
half the HBM traffic of fp32); accumulation is always fp32 in PSUM.

Phase order is chosen for DMA/compute overlap from a cold start:
  Q projection first (needs only 1.3 MB before the first matmul), then V, then
  K^T, then attention+projection. Phase B defers pair P's output projection
  until after pair P+1's score/PV matmuls so PSUM->SBUF copies are hidden.
"""

import numpy as np
import ml_dtypes

import concourse.bass as bass
import concourse.tile as tile
import concourse.mybir as mybir
from concourse import bacc
from concourse.bass import ts, ds
from concourse.bass_utils import run_bass_kernel_spmd

F32 = mybir.dt.float32
F16 = mybir.dt.float16

T, D = 2048, 1024
NT = T // 128          # 16 query/key tiles
DC = D // 128          # 8 contraction chunks
# own query tiles per core parity (descending pairing balances causal work)
# pair P of parity0: (CP[P]-1, CP[P]-4); parity1: (CP[P]-2, CP[P]-3)
OWN = [[15, 12, 11, 8, 7, 4, 3, 0],
       [14, 13, 10, 9, 6, 5, 2, 1]]
CP = [16, 12, 8, 4]    # j-blocks per slot-pair (uniform across cores)

_NC_CACHE = {}


def _build(repeat=1, phases=3):
    key = (repeat, phases)
    if key in _NC_CACHE:
        return _NC_CACHE[key]
    nc = bacc.Bacc("TRN2", target_bir_lowering=False, debug=False,
                   enable_asserts=False, num_devices=8)
    xT = nc.dram_tensor("xT", [D, T], F16, kind="ExternalInput").ap()
    xqT = nc.dram_tensor("xqT", [D, 1024], F16, kind="ExternalInput").ap()
    # wq/wk are host-pre-rearranged to the SBUF stationary layout:
    # wq[m][p][c*128+f] = W[c*128+p][m*128+f] so each m-tile is one
    # contiguous 256 KB DMA instead of a 1024-line scatter.
    wq = nc.dram_tensor("wq", [DC, 128, D], F16, kind="ExternalInput").ap()
    wk = nc.dram_tensor("wk", [DC, 128, D], F16, kind="ExternalInput").ap()
    wv = nc.dram_tensor("wv", [D, D], F16, kind="ExternalInput").ap()
    wp = nc.dram_tensor("wp", [D, D], F16, kind="ExternalInput").ap()
    bq = nc.dram_tensor("bq", [D], F32, kind="ExternalInput").ap()
    # per pair: masks256 [2,128,256] (tj=cp-4,cp-3), masks128 [2,128,128]
    m256 = nc.dram_tensor("m256", [4, 2, 128, 256], F16, kind="ExternalInput").ap()
    m128 = nc.dram_tensor("m128", [4, 2, 128, 128], F16, kind="ExternalInput").ap()
    out = nc.dram_tensor("out", [1024, D], F16, kind="ExternalOutput").ap()
    den_dram = nc.dram_tensor("den_scratch", [1024], F32).ap()

    with tile.TileContext(nc, pool_alloc_mode="queue") as tc:
        def body(_i=None):
            _emit(nc, tc, xT, xqT, wq, wk, wv, wp, bq, m256, m128, out,
                  den_dram, phases)
        if repeat == 1:
            body()
        else:
            with tc.For_i(0, repeat, 1):
                body()
    nc.compile()
    _NC_CACHE[key] = nc
    return nc


def _emit(nc, tc, xT, xqT, wq, wk, wv, wp, bq, m256, m128, out, den_dram,
          phases=3):
    with tc.tile_pool(name="pk", bufs=1) as pk_pool, \
         tc.tile_pool(name="pv", bufs=1) as pv_pool, \
         tc.tile_pool(name="pq", bufs=1) as pq_pool, \
         tc.tile_pool(name="wvp", bufs=1) as wv_pool, \
         tc.tile_pool(name="wkp", bufs=1) as wk_pool, \
         tc.tile_pool(name="wpp", bufs=1) as wp_pool, \
         tc.tile_pool(name="mskp", bufs=1) as msk_pool, \
         tc.tile_pool(name="small", bufs=1) as small:

        # ---- Phase Q: Q^T (own rows; needs only xqT + wq) ----
        # ic-outer so the first matmul needs only wqm[0] + xq[*][0] (~1.3 MB)
        bq_sb = small.tile([128, 8], F32, tag="bq", name="bq_sb")
        qT_sb = [[pq_pool.tile([128, 256], F16, tag=f"q{m}_{p}", name=f"qT_sb{m}_{p}")
                  for p in range(4)] for m in range(DC)]
        # one PSUM pool shared by phases Q/V/K (same tile shape) — avoids
        # pool-boundary syncs between the projection phases
        psA_cm = tc.tile_pool(name="psA", bufs=6, space="PSUM")
        psA = psA_cm.__enter__()
        with tc.tile_pool(name="xq", bufs=1) as xq_pool, \
             tc.tile_pool(name="wqm", bufs=1) as wq_pool:
            xq = [[xq_pool.tile([128, 512], F16, tag=f"xq{d}_{j}", name=f"xq{d}_{j}")
                   for j in range(2)] for d in range(DC)]
            wqm = [wq_pool.tile([128, 1024], F16, tag=f"wqm{m}", name=f"wqm{m}")
                   for m in range(DC)]

            def load_wqm(m):
                nc.sync.dma_start(wqm[m][:], wq[m, :, :])

            # DMA emission order = consumption order of the cold-start
            # m-group sweep below (the queue is serial).
            def wqm_half(m, h):
                nc.sync.dma_start(wqm[m][:, ds(512 * h, 512)],
                                  wq[m, :, ds(512 * h, 512)])

            wqm_half(0, 0)
            nc.sync.dma_start(xq[0][0][:], xqT[ts(0, 128), ts(0, 512)])
            for m in (1, 2, 3):
                wqm_half(m, 0)
            nc.sync.dma_start(bq_sb[:], bq.rearrange("(m p) -> p m", p=128))
            nc.sync.dma_start(xq[1][0][:], xqT[ts(1, 128), ts(0, 512)])
            wqm_half(0, 1)
            nc.sync.dma_start(xq[2][0][:], xqT[ts(2, 128), ts(0, 512)])
            wqm_half(1, 1)
            nc.sync.dma_start(xq[3][0][:], xqT[ts(3, 128), ts(0, 512)])
            for m in (2, 3):
                wqm_half(m, 1)
            for d in range(4, DC):
                nc.sync.dma_start(xq[d][0][:], xqT[ts(d, 128), ts(0, 512)])
            for m in range(4, DC):
                load_wqm(m)
            for d in range(DC):
                nc.sync.dma_start(xq[d][1][:], xqT[ts(d, 128), ts(1, 512)])

            if True:
                # ic=0 runs while xq/wqm stream in: m-groups of 4 with d
                # outer, so each arriving 128 KB xq chunk funds 4 matmuls —
                # compute density matches the DMA arrival rate from cold.
                for mg in (0, 4):
                    pss = [psA.tile([128, 512], F32, tag="A", name="psQ_t")
                           for _ in range(4)]
                    for d in range(DC):
                        for mi in range(4):
                            nc.tensor.matmul(pss[mi][:],
                                             wqm[mg + mi][:, ts(d, 128)],
                                             xq[d][0][:],
                                             start=(d == 0), stop=(d == DC - 1))
                    for mi in range(4):
                        for p2 in range(2):
                            nc.scalar.activation(qT_sb[mg + mi][p2][:],
                                                 pss[mi][:, ts(p2, 256)],
                                                 mybir.ActivationFunctionType.Identity,
                                                 bias=bq_sb[:, mg + mi:mg + mi + 1])
                for m in range(DC):
                    ps = psA.tile([128, 512], F32, tag="A", name="psQ_t")
                    for d in range(DC):
                        nc.tensor.matmul(ps[:],
                                         wqm[m][:, ts(d, 128)],
                                         xq[d][1][:],
                                         start=(d == 0), stop=(d == DC - 1))
                    for p2 in range(2):
                        nc.scalar.activation(qT_sb[m][2 + p2][:],
                                             ps[:, ts(p2, 256)],
                                             mybir.ActivationFunctionType.Identity,
                                             bias=bq_sb[:, m:m + 1])

        # ---- xT arrives while Q computes; wv before xt so V can start ----
        with tc.tile_pool(name="xt", bufs=1) as xt_pool:
            wv_sb = [[wv_pool.tile([128, 512], F16, tag=f"wv{fc}_{d}",
                                   name=f"wv_sb{fc}_{d}") for d in range(DC)]
                     for fc in range(2)]
            for d in range(DC):
                nc.sync.dma_start(wv_sb[0][d][:], wv[ts(d, 128), ts(0, 512)])
            xt = [[xt_pool.tile([128, 512], F16, tag=f"xt{d}_{j}", name=f"xt{d}_{j}")
                   for j in range(4)] for d in range(DC)]
            for j in range(4):
                for d in range(DC):
                    nc.sync.dma_start(xt[d][j][:], xT[ts(d, 128), ts(j, 512)])
            for d in range(DC):
                nc.sync.dma_start(wv_sb[1][d][:], wv[ts(d, 128), ts(1, 512)])
            # K-phase weights next (needed before wp/masks; the DMA queue is
            # serial, so order = need order), then B-phase prefetch.
            wkm = [wk_pool.tile([128, 1024], F16, tag=f"wkm{m}", name=f"wkm{m}")
                   for m in range(DC)]
            for m in range(DC):
                nc.sync.dma_start(wkm[m][:], wk[m, :, :])
            wp_sb = [[wp_pool.tile([128, 512], F16, tag=f"wp{d}_{f}",
                                   name=f"wp_sb{d}_{f}") for f in range(2)]
                     for d in range(DC)]
            msk256 = [[msk_pool.tile([128, 256], F16, tag=f"m256_{P}_{i}",
                                     name=f"m256_{P}_{i}") for i in range(2)]
                      for P in range(4)]
            msk128 = [[msk_pool.tile([128, 128], F16, tag=f"m128_{P}_{i}",
                                     name=f"m128_{P}_{i}") for i in range(2)]
                      for P in range(4)]
            for d in range(DC):
                for f in range(2):
                    nc.sync.dma_start(wp_sb[d][f][:], wp[ts(d, 128), ts(f, 512)])
            for P in range(4):
                for i in range(2):
                    nc.sync.dma_start(msk256[P][i][:], m256[P, i, :, :])
                    nc.sync.dma_start(msk128[P][i][:], m128[P, i, :, :])

            # ---- Phase V: V = X @ Wv/8 (full batch) ----
            v_sb = [pv_pool.tile([128, D], F16, tag=f"v{t}", name=f"v_sb{t}")
                    for t in range(NT)]
            if True:
                for fc in range(2):
                    for tt in range(NT):
                        ps = psA.tile([128, 512], F32, tag="A", name="psV_t")
                        for d in range(DC):
                            nc.tensor.matmul(ps[:],
                                             xt[d][tt // 4][:, ts(tt % 4, 128)],
                                             wv_sb[fc][d][:],
                                             start=(d == 0), stop=(d == DC - 1))
                        nc.vector.tensor_copy(v_sb[tt][:, ts(fc, 512)], ps[:])

            # ---- Phase K: K^T (needs all of xT) ----
            kT_sb = [pk_pool.tile([128, T], F16, tag=f"k{m}", name=f"kT_sb{m}")
                     for m in range(DC)]
            if True:
                for m in range(DC):
                    for jc in range(4):
                        # one PSUM tile per 512-token block: its copy starts
                        # as soon as the block's d-loop finishes, so only the
                        # last block's copy remains at the K->B boundary.
                        ps = psA.tile([128, 512], F32, tag="A", name="psK_t")
                        for d in range(DC):
                            nc.tensor.matmul(ps[:],
                                             wkm[m][:, ts(d, 128)],
                                             xt[d][jc][:],
                                             start=(d == 0), stop=(d == DC - 1))
                        nc.scalar.copy(kT_sb[m][:, ts(jc, 512)], ps[:])

        psA_cm.__exit__(None, None, None)

        if phases <= 1:
            with tc.tile_pool(name="dump", bufs=1) as dump:
                tk = dump.tile([128, 512], F16, tag="tk", name="tk")
                nc.vector.tensor_copy(tk[:], kT_sb[0][:, 0:512])
                nc.sync.dma_start(out[0:128, 0:512], tk[:])
                tq = dump.tile([128, 512], F16, tag="tq", name="tq")
                nc.vector.tensor_copy(tq[:, 0:256], qT_sb[0][0][:])
                nc.sync.dma_start(out[0:128, 512:1024], tq[:])
                tv = dump.tile([128, 512], F16, tag="tv", name="tv")
                nc.vector.tensor_copy(tv[:], v_sb[0][:, 0:512])
                nc.sync.dma_start(out[128:256, 0:512], tv[:])
            return

        # ---- Phase B: attention + projection ----
        with tc.tile_pool(name="transB", bufs=3) as trans, \
             tc.tile_pool(name="po", bufs=1, space="PSUM") as po_pool, \
             tc.tile_pool(name="psS", bufs=2, space="PSUM") as psS_pool, \
             tc.tile_pool(name="pden", bufs=1, space="PSUM") as pden_pool, \
             tc.tile_pool(name="pproj", bufs=2, space="PSUM") as pp_pool:

            ones_bf = small.tile([128, 1], F16, tag="ones", name="ones_bf")
            nc.vector.memset(ones_bf[:], 1.0)

            oT_all = {}
            recip_all = {}

            def width(P, tj):
                return 256 if tj < CP[P] - 2 else 128

            def emit_scores(P, proj_units=()):
                """Pure score-matmul burst (exp/mask on ACT/DVE run behind);
                proj_units for pair P-1 are interleaved at spread points."""
                cp = CP[P]
                proj_at = {}
                for g, unit in enumerate(proj_units):
                    proj_at.setdefault(max(0, cp - 1 - 2 * g), []).append(unit)
                pts = []
                for tj in range(cp):
                    w = width(P, tj)
                    psS = psS_pool.tile([128, 256], F32, tag="s", name="psS_t")
                    for d in range(DC):
                        nc.tensor.matmul(psS[:, 0:w],
                                         kT_sb[d][:, ts(tj, 128)],
                                         qT_sb[d][P][:, 0:w],
                                         start=(d == 0), stop=(d == DC - 1))
                    pt = trans.tile([128, 256], F16, tag=f"pt{tj}", name="pt_t",
                                    bufs=2)
                    nc.scalar.activation(pt[:, 0:w], psS[:, 0:w],
                                         mybir.ActivationFunctionType.Exp)
                    mi = tj - (cp - 4)
                    if mi >= 0:
                        if w == 256:
                            nc.vector.tensor_mul(pt[:], pt[:], msk256[P][mi][:])
                        else:
                            nc.vector.tensor_mul(pt[:, 0:128], pt[:, 0:128],
                                                 msk128[P][mi - 2][:])
                    pts.append((pt, w))
                    for unit in proj_at.get(tj, ()):
                        unit()
                return pts

            def emit_den_pv(P, pts, units=()):
                cp = CP[P]
                # den burst first: all exps have retired by now, so these
                # matmuls never make the PE wait on the ACT/DVE chain.
                pden = pden_pool.tile([1, 256], F32, tag="den", name="pden_t")
                for tj in range(cp):
                    pt, w = pts[tj]
                    nc.tensor.matmul(pden[:, 0:w], ones_bf[:], pt[:, 0:w],
                                     start=(tj == 0), stop=(tj == cp - 1))
                den_row = trans.tile([1, 256], F32, tag="denrow", name="den_row")
                nc.vector.tensor_copy(den_row[:], pden[:])
                nc.sync.dma_start(den_dram[ds(256 * P, 256)], den_row[:])
                den_col = trans.tile([128, 2], F32, tag="dencol", name="den_col",
                                     bufs=2)
                nc.sync.dma_start(den_col[:],
                                  den_dram[ds(256 * P, 256)]
                                  .rearrange("(t p) -> p t", p=128))
                recip = trans.tile([128, 2], F32, tag="recip", name="recip",
                                   bufs=2)
                nc.vector.reciprocal(recip[:], den_col[:])
                recip_all[P] = recip

                if phases <= 2:
                    return
                # spill-over proj units from a cramped (small-cp) scores
                # burst run here, BEFORE pvt is allocated so the pp-pool
                # rotation never waits on the later-emitted oT copy
                for unit in units:
                    unit()
                # single-pass PV: 8 dt accumulation groups packed 2-per-bank;
                # the odd-dt group's first matmul lands on a bank its even
                # sibling already cleared (has_written=0 -> overwrite).
                # po holds dt 0-5 (3 banks); dt 6-7 live in a pp-pool tile,
                # which is idle during PV — this frees a PSUM bank so the
                # projection pool can double-buffer.
                po = po_pool.tile([128, 1536], F32, tag="o", name="po_t")
                pvt = pp_pool.tile([128, 512], F32, tag="pp", name="pv_tail")
                oT = trans.tile([128, 2048], F16, tag="oT", name="oT_t", bufs=2)

                def po_slice(dt, w):
                    if dt < 6:
                        return po[:, ds(256 * dt, w)]
                    return pvt[:, ds(256 * (dt - 6), w)]

                # PV in two dt-halves so each half's oT copy overlaps the
                # other half's matmuls — the last pair's copy (critical path
                # into its projection) is mostly hidden.
                for half in range(2):
                    for tj in range(cp):
                        pt, w = pts[tj]
                        for dtl in range(4):
                            dt = 4 * half + dtl
                            nc.tensor.matmul(po_slice(dt, w),
                                             v_sb[tj][:, ts(dt, 128)],
                                             pt[:, 0:w],
                                             start=(tj == 0 and dt % 2 == 0),
                                             stop=(tj == cp - 1),
                                             skip_group_check=True)
                    if half == 0:
                        nc.vector.tensor_copy(oT[:, 0:512], po[:, 0:512])
                        nc.scalar.copy(oT[:, 512:1024], po[:, 512:1024])
                    else:
                        nc.vector.tensor_copy(oT[:, 1024:1536], po[:, 1024:1536])
                        nc.scalar.copy(oT[:, 1536:2048], pvt[:, 0:512])
                oT_all[P] = oT

            def proj_unit(P, it, fo):
                def unit():
                    pp = pp_pool.tile([128, 512], F32, tag="pp", name="pp_t")
                    for dt in range(DC):
                        nc.tensor.matmul(pp[:],
                                         oT_all[P][:, ds(256 * dt + 128 * it, 128)],
                                         wp_sb[dt][fo][:],
                                         start=(dt == 0), stop=(dt == DC - 1))
                    ob = trans.tile([128, 512], F16, tag="ob", name="ob_t")
                    nc.vector.tensor_scalar_mul(ob[:], pp[:],
                                                recip_all[P][:, it:it + 1])
                    nc.sync.dma_start(
                        out[ds(128 * (2 * P + it), 128), ts(fo, 512)], ob[:])
                return unit

            def proj_units(P):
                return [proj_unit(P, it, fo) for it in range(2) for fo in range(2)]

            def proj_unit_half(P, it, fo, h):
                def unit():
                    pp = pp_pool.tile([128, 512], F32, tag="pp", name="pp_t")
                    for dt in range(DC):
                        nc.tensor.matmul(pp[:, 0:256],
                                         oT_all[P][:, ds(256 * dt + 128 * it, 128)],
                                         wp_sb[dt][fo][:, ds(256 * h, 256)],
                                         start=(dt == 0), stop=(dt == DC - 1))
                    ob = trans.tile([128, 256], F16, tag="obh", name="obh_t",
                                    bufs=2)
                    nc.vector.tensor_scalar_mul(ob[:], pp[:, 0:256],
                                                recip_all[P][:, it:it + 1])
                    nc.sync.dma_start(
                        out[ds(128 * (2 * P + it), 128),
                            ds(512 * fo + 256 * h, 256)], ob[:])
                return unit

            for P in range(4):
                units = proj_units(P - 1) if (phases > 2 and P >= 1) else ()
                if CP[P] < 8:
                    pts = emit_scores(P, units[:2])
                    emit_den_pv(P, pts, units[2:])
                else:
                    pts = emit_scores(P, units)
                    emit_den_pv(P, pts)
            if phases > 2:
                # last pair's projection: final unit split in half so the
                # end-of-kernel scale+DMA chain is half as long
                tail = proj_units(3)
                for unit in tail[:3]:
                    unit()
                proj_unit_half(3, 1, 1, 0)()
                proj_unit_half(3, 1, 1, 1)()


def _host_masks(par):
    """Uniform-template masks.

    m256[P, i] multiplies the P tile at tj = cp-4+i (full 256-wide blocks);
    m128[P, i] multiplies the slot0 half at tj = cp-2+i (128-wide blocks).
    parity0 pair P owns (hi, lo) = (cp-1, cp-4); parity1 owns (cp-2, cp-3).
    """
    m256 = np.zeros((4, 2, 128, 256), np.float32)
    m128 = np.zeros((4, 2, 128, 128), np.float32)
    j = np.arange(128)[:, None]
    i = np.arange(128)[None, :]
    tri = (j <= i).astype(np.float32)   # diagonal tile mask
    ones = np.ones((128, 128), np.float32)
    zeros = np.zeros((128, 128), np.float32)
    for P in range(4):
        cp = CP[P]
        if par == 0:
            # hi = cp-1 at slot0, lo = cp-4 at slot1
            m256[P, 0, :, 0:128] = ones   # tj=cp-4 vs hi: below diag
            m256[P, 0, :, 128:256] = tri  # tj=cp-4 == lo: diagonal
            m256[P, 1, :, 0:128] = ones   # tj=cp-3 vs hi: below diag
            m256[P, 1, :, 128:256] = zeros  # tj=cp-3 > lo: dead
            m128[P, 0] = ones             # tj=cp-2 < hi
            m128[P, 1] = tri              # tj=cp-1 == hi: diagonal
        else:
            # hi = cp-2 at slot0, lo = cp-3 at slot1
            m256[P, 0, :, 0:128] = ones   # tj=cp-4 < hi
            m256[P, 0, :, 128:256] = ones  # tj=cp-4 < lo
            m256[P, 1, :, 0:128] = ones   # tj=cp-3 < hi
            m256[P, 1, :, 128:256] = tri  # tj=cp-3 == lo: diagonal
            m128[P, 0] = tri              # tj=cp-2 == hi: diagonal
            m128[P, 1] = zeros            # tj=cp-1 > hi: dead
    return m256.astype(np.float16), m128.astype(np.float16)


def kernel(x, W_attn, b_attn, W_proj, b_proj, _repeat=1, _results_only=False,
           _phases=3):
    x = np.asarray(x, np.float32)
    W_attn = np.asarray(W_attn, np.float32)
    b_attn = np.asarray(b_attn, np.float32)
    W_proj = np.asarray(W_proj, np.float32)
    b_proj = np.asarray(b_proj, np.float32)
    B = x.shape[0]

    nc = _build(_repeat, _phases)

    b_eff = (b_proj.astype(np.float64)
             + b_attn[2 * D:].astype(np.float64) @ W_proj.astype(np.float64)
             ).astype(np.float32)
    # exact pow-2 rescales: q,bq /8 (softmax scale); v /8 and wp *8 (fp16 range)
    def stat_layout(w):
        # [D, D] -> [m, p, c*128+f] with w[c*128+p][m*128+f]
        return np.ascontiguousarray(
            w.reshape(DC, 128, DC, 128).transpose(2, 1, 0, 3).reshape(DC, 128, D))

    wq = stat_layout((W_attn[:, :D] * np.float32(0.125)).astype(np.float16))
    wk = stat_layout(W_attn[:, D:2 * D].astype(np.float16))
    wv = (W_attn[:, 2 * D:] * np.float32(0.125)).astype(np.float16)
    wp = (W_proj * np.float32(8.0)).astype(np.float16)
    bqv = (b_attn[:D] * np.float32(0.125)).astype(np.float32)
    masks_by_par = [_host_masks(0), _host_masks(1)]

    in_maps = []
    for c in range(8):
        b, par = c // 2, c % 2
        own = OWN[par]
        xTb = np.ascontiguousarray(x[b].T.astype(np.float16))
        cols = np.concatenate([np.arange(128 * t, 128 * (t + 1)) for t in own])
        xqT = np.ascontiguousarray(xTb[:, cols])
        m256, m128 = masks_by_par[par]
        in_maps.append({"xT": xTb, "xqT": xqT, "wq": wq, "wk": wk, "wv": wv,
                        "wp": wp, "bq": bqv, "m256": m256, "m128": m128})

    res = run_bass_kernel_spmd(nc, in_maps, core_ids=list(range(8)))
    if _results_only:
        return res

    out = np.empty((B, T, D), np.float32)
    for c in range(8):
        b, par = c // 2, c % 2
        part = res.results[c]["out"].astype(np.float32)
        for s, t in enumerate(OWN[par]):
            out[b, 128 * t:128 * (t + 1), :] = part[128 * s:128 * (s + 1), :] + b_eff
    return out


# revision 46
# speedup vs baseline: 1.1198x; 1.1198x over previous
"""Trainium2 Bass kernel for nn_CausalSelfAttention_8237747274097.

Reference math (single-head attention over full n_embd=1024, scale 1/8):
    qkv = x @ W_attn + b_attn ; q,k,v = split(qkv)
    att = softmax(causal(q @ k.T / 8)) ; y = att @ v ; out = y @ W_proj + b_proj

Sharding (8 cores): core c = (batch b = c//2, parity p = c%2). Each core owns 8
of the 16 query row-tiles (128 rows each) of its batch, interleaved/paired so
causal work is balanced, and computes full K/V for the batch. Outputs are
disjoint row slices -> host gather is a pure scatter + bias add.

Math simplifications (all exact):
  - k bias drops out of softmax; v bias folds into b_eff = b_proj + b_v@W_proj.
  - 1/8 q scale folded into W_q/b_q; V scaled 1/8 and W_proj scaled 8 (exact
    pow-2) so fp16-stored attention numerators stay far from fp16 overflow.
Softmax is computed without max-subtraction (scores are O(3); exp is safe) so
the denominator comes free from a ones-row matmul.

Precision: all matmul operands fp16 (full PE rate, FWL-eligible weight loads,
half the HBM traffic of fp32); accumulation is always fp32 in PSUM.

Phase order is chosen for DMA/compute overlap from a cold start:
  Q projection first (needs only 1.3 MB before the first matmul), then V, then
  K^T, then attention+projection. Phase B defers pair P's output projection
  until after pair P+1's score/PV matmuls so PSUM->SBUF copies are hidden.
"""

import numpy as np
import ml_dtypes

import concourse.bass as bass
import concourse.tile as tile
import concourse.mybir as mybir
from concourse import bacc
from concourse.bass import ts, ds
from concourse.bass_utils import run_bass_kernel_spmd

F32 = mybir.dt.float32
F16 = mybir.dt.float16

T, D = 2048, 1024
NT = T // 128          # 16 query/key tiles
DC = D // 128          # 8 contraction chunks
# own query tiles per core parity (descending pairing balances causal work)
# pair P of parity0: (CP[P]-1, CP[P]-4); parity1: (CP[P]-2, CP[P]-3)
OWN = [[15, 12, 11, 8, 7, 4, 3, 0],
       [14, 13, 10, 9, 6, 5, 2, 1]]
CP = [16, 12, 8, 4]    # j-blocks per slot-pair (uniform across cores)

_NC_CACHE = {}


def _build(repeat=1, phases=3):
    key = (repeat, phases)
    if key in _NC_CACHE:
        return _NC_CACHE[key]
    nc = bacc.Bacc("TRN2", target_bir_lowering=False, debug=False,
                   enable_asserts=False, num_devices=8)
    xT = nc.dram_tensor("xT", [D, T], F16, kind="ExternalInput").ap()
    xqT = nc.dram_tensor("xqT", [D, 1024], F16, kind="ExternalInput").ap()
    # wq/wk are host-pre-rearranged to the SBUF stationary layout:
    # wq[m][p][c*128+f] = W[c*128+p][m*128+f] so each m-tile is one
    # contiguous 256 KB DMA instead of a 1024-line scatter.
    wq = nc.dram_tensor("wq", [DC, 128, D], F16, kind="ExternalInput").ap()
    wk = nc.dram_tensor("wk", [DC, 128, D], F16, kind="ExternalInput").ap()
    # wv carries Wv @ W_proj (host-folded): PV's output is then the final
    # (pre-bias, pre-1/den) result and no on-device projection is needed.
    wv = nc.dram_tensor("wv", [D, D], F16, kind="ExternalInput").ap()
    bq = nc.dram_tensor("bq", [D], F32, kind="ExternalInput").ap()
    # per pair: masks256 [2,128,256] (tj=cp-4,cp-3), masks128 [2,128,128]
    m256 = nc.dram_tensor("m256", [4, 2, 128, 256], F16, kind="ExternalInput").ap()
    m128 = nc.dram_tensor("m128", [4, 2, 128, 128], F16, kind="ExternalInput").ap()
    out = nc.dram_tensor("out", [1024, D], F16, kind="ExternalOutput").ap()
    den_dram = nc.dram_tensor("den_scratch", [1024], F32).ap()

    with tile.TileContext(nc, pool_alloc_mode="queue") as tc:
        def body(_i=None):
            _emit(nc, tc, xT, xqT, wq, wk, wv, bq, m256, m128, out,
                  den_dram, phases)
        if repeat == 1:
            body()
        else:
            with tc.For_i(0, repeat, 1):
                body()
    nc.compile()
    _NC_CACHE[key] = nc
    return nc


def _emit(nc, tc, xT, xqT, wq, wk, wv, bq, m256, m128, out, den_dram,
          phases=3):
    with tc.tile_pool(name="pk", bufs=1) as pk_pool, \
         tc.tile_pool(name="pv", bufs=1) as pv_pool, \
         tc.tile_pool(name="pq", bufs=1) as pq_pool, \
         tc.tile_pool(name="wvp", bufs=1) as wv_pool, \
         tc.tile_pool(name="wkp", bufs=1) as wk_pool, \
         tc.tile_pool(name="mskp", bufs=1) as msk_pool, \
         tc.tile_pool(name="small", bufs=1) as small:

        # ---- Phase Q: Q^T (own rows; needs only xqT + wq) ----
        # ic-outer so the first matmul needs only wqm[0] + xq[*][0] (~1.3 MB)
        bq_sb = small.tile([128, 8], F32, tag="bq", name="bq_sb")
        qT_sb = [[pq_pool.tile([128, 256], F16, tag=f"q{m}_{p}", name=f"qT_sb{m}_{p}")
                  for p in range(4)] for m in range(DC)]
        # one PSUM pool shared by phases Q/V/K (same tile shape) — avoids
        # pool-boundary syncs between the projection phases
        psA_cm = tc.tile_pool(name="psA", bufs=6, space="PSUM")
        psA = psA_cm.__enter__()
        with tc.tile_pool(name="xq", bufs=1) as xq_pool, \
             tc.tile_pool(name="wqm", bufs=1) as wq_pool:
            xq = [[xq_pool.tile([128, 512], F16, tag=f"xq{d}_{j}", name=f"xq{d}_{j}")
                   for j in range(2)] for d in range(DC)]
            wqm = [wq_pool.tile([128, 1024], F16, tag=f"wqm{m}", name=f"wqm{m}")
                   for m in range(DC)]

            def load_wqm(m):
                nc.sync.dma_start(wqm[m][:], wq[m, :, :])

            # DMA emission order = consumption order of the cold-start
            # m-group sweep below (the queue is serial).
            def wqm_half(m, h):
                nc.sync.dma_start(wqm[m][:, ds(512 * h, 512)],
                                  wq[m, :, ds(512 * h, 512)])

            wqm_half(0, 0)
            nc.sync.dma_start(xq[0][0][:], xqT[ts(0, 128), ts(0, 512)])
            for m in (1, 2, 3):
                wqm_half(m, 0)
            nc.sync.dma_start(bq_sb[:], bq.rearrange("(m p) -> p m", p=128))
            nc.sync.dma_start(xq[1][0][:], xqT[ts(1, 128), ts(0, 512)])
            wqm_half(0, 1)
            nc.sync.dma_start(xq[2][0][:], xqT[ts(2, 128), ts(0, 512)])
            wqm_half(1, 1)
            nc.sync.dma_start(xq[3][0][:], xqT[ts(3, 128), ts(0, 512)])
            for m in (2, 3):
                wqm_half(m, 1)
            for d in range(4, DC):
                nc.sync.dma_start(xq[d][0][:], xqT[ts(d, 128), ts(0, 512)])
            for m in range(4, DC):
                load_wqm(m)
            for d in range(DC):
                nc.sync.dma_start(xq[d][1][:], xqT[ts(d, 128), ts(1, 512)])

            if True:
                # ic=0 runs while xq/wqm stream in: m-groups of 4 with d
                # outer, so each arriving 128 KB xq chunk funds 4 matmuls —
                # compute density matches the DMA arrival rate from cold.
                for mg in (0, 4):
                    pss = [psA.tile([128, 512], F32, tag="A", name="psQ_t")
                           for _ in range(4)]
                    for d in range(DC):
                        for mi in range(4):
                            nc.tensor.matmul(pss[mi][:],
                                             wqm[mg + mi][:, ts(d, 128)],
                                             xq[d][0][:],
                                             start=(d == 0), stop=(d == DC - 1))
                    for mi in range(4):
                        for p2 in range(2):
                            nc.scalar.activation(qT_sb[mg + mi][p2][:],
                                                 pss[mi][:, ts(p2, 256)],
                                                 mybir.ActivationFunctionType.Identity,
                                                 bias=bq_sb[:, mg + mi:mg + mi + 1])
                for m in range(DC):
                    ps = psA.tile([128, 512], F32, tag="A", name="psQ_t")
                    for d in range(DC):
                        nc.tensor.matmul(ps[:],
                                         wqm[m][:, ts(d, 128)],
                                         xq[d][1][:],
                                         start=(d == 0), stop=(d == DC - 1))
                    for p2 in range(2):
                        nc.scalar.activation(qT_sb[m][2 + p2][:],
                                             ps[:, ts(p2, 256)],
                                             mybir.ActivationFunctionType.Identity,
                                             bias=bq_sb[:, m:m + 1])

        # ---- xT arrives while Q computes; wv before xt so V can start ----
        with tc.tile_pool(name="xt", bufs=1) as xt_pool:
            wv_sb = [[wv_pool.tile([128, 512], F16, tag=f"wv{fc}_{d}",
                                   name=f"wv_sb{fc}_{d}") for d in range(DC)]
                     for fc in range(2)]
            for d in range(DC):
                nc.sync.dma_start(wv_sb[0][d][:], wv[ts(d, 128), ts(0, 512)])
            xt = [[xt_pool.tile([128, 512], F16, tag=f"xt{d}_{j}", name=f"xt{d}_{j}")
                   for j in range(4)] for d in range(DC)]
            for j in range(4):
                for d in range(DC):
                    nc.sync.dma_start(xt[d][j][:], xT[ts(d, 128), ts(j, 512)])
            for d in range(DC):
                nc.sync.dma_start(wv_sb[1][d][:], wv[ts(d, 128), ts(1, 512)])
            # K-phase weights next (needed before wp/masks; the DMA queue is
            # serial, so order = need order), then B-phase prefetch.
            wkm = [wk_pool.tile([128, 1024], F16, tag=f"wkm{m}", name=f"wkm{m}")
                   for m in range(DC)]
            for m in range(DC):
                nc.sync.dma_start(wkm[m][:], wk[m, :, :])
            msk256 = [[msk_pool.tile([128, 256], F16, tag=f"m256_{P}_{i}",
                                     name=f"m256_{P}_{i}") for i in range(2)]
                      for P in range(4)]
            msk128 = [[msk_pool.tile([128, 128], F16, tag=f"m128_{P}_{i}",
                                     name=f"m128_{P}_{i}") for i in range(2)]
                      for P in range(4)]
            for P in range(4):
                for i in range(2):
                    nc.sync.dma_start(msk256[P][i][:], m256[P, i, :, :])
                    nc.sync.dma_start(msk128[P][i][:], m128[P, i, :, :])

            # ---- Phase V: V = X @ Wv/8 (full batch) ----
            v_sb = [pv_pool.tile([128, D], F16, tag=f"v{t}", name=f"v_sb{t}")
                    for t in range(NT)]
            if True:
                for fc in range(2):
                    for tt in range(NT):
                        ps = psA.tile([128, 512], F32, tag="A", name="psV_t")
                        for d in range(DC):
                            nc.tensor.matmul(ps[:],
                                             xt[d][tt // 4][:, ts(tt % 4, 128)],
                                             wv_sb[fc][d][:],
                                             start=(d == 0), stop=(d == DC - 1))
                        nc.vector.tensor_copy(v_sb[tt][:, ts(fc, 512)], ps[:])

            # ---- Phase K: K^T (needs all of xT) ----
            kT_sb = [pk_pool.tile([128, T], F16, tag=f"k{m}", name=f"kT_sb{m}")
                     for m in range(DC)]
            if True:
                for m in range(DC):
                    for jc in range(4):
                        # one PSUM tile per 512-token block: its copy starts
                        # as soon as the block's d-loop finishes, so only the
                        # last block's copy remains at the K->B boundary.
                        ps = psA.tile([128, 512], F32, tag="A", name="psK_t")
                        for d in range(DC):
                            nc.tensor.matmul(ps[:],
                                             wkm[m][:, ts(d, 128)],
                                             xt[d][jc][:],
                                             start=(d == 0), stop=(d == DC - 1))
                        nc.scalar.copy(kT_sb[m][:, ts(jc, 512)], ps[:])

        psA_cm.__exit__(None, None, None)

        if phases <= 1:
            with tc.tile_pool(name="dump", bufs=1) as dump:
                tk = dump.tile([128, 512], F16, tag="tk", name="tk")
                nc.vector.tensor_copy(tk[:], kT_sb[0][:, 0:512])
                nc.sync.dma_start(out[0:128, 0:512], tk[:])
                tq = dump.tile([128, 512], F16, tag="tq", name="tq")
                nc.vector.tensor_copy(tq[:, 0:256], qT_sb[0][0][:])
                nc.sync.dma_start(out[0:128, 512:1024], tq[:])
                tv = dump.tile([128, 512], F16, tag="tv", name="tv")
                nc.vector.tensor_copy(tv[:], v_sb[0][:, 0:512])
                nc.sync.dma_start(out[128:256, 0:512], tv[:])
            return

        # ---- Phase B: attention + projection ----
        with tc.tile_pool(name="transB", bufs=3) as trans, \
             tc.tile_pool(name="poF", bufs=1, space="PSUM") as po_pool, \
             tc.tile_pool(name="psS", bufs=2, space="PSUM") as psS_pool, \
             tc.tile_pool(name="pden", bufs=1, space="PSUM") as pden_pool:

            ones_bf = small.tile([128, 1], F16, tag="ones", name="ones_bf")
            nc.vector.memset(ones_bf[:], 1.0)

            recip_all = {}

            def width(P, tj):
                return 256 if tj < CP[P] - 2 else 128

            def emit_scores(P, proj_units=()):
                """Pure score-matmul burst (exp/mask on ACT/DVE run behind);
                proj_units for pair P-1 are interleaved at spread points."""
                cp = CP[P]
                proj_at = {}
                for g, unit in enumerate(proj_units):
                    proj_at.setdefault(max(0, cp - 1 - 2 * g), []).append(unit)
                pts = []
                for tj in range(cp):
                    w = width(P, tj)
                    psS = psS_pool.tile([128, 256], F32, tag="s", name="psS_t")
                    for d in range(DC):
                        nc.tensor.matmul(psS[:, 0:w],
                                         kT_sb[d][:, ts(tj, 128)],
                                         qT_sb[d][P][:, 0:w],
                                         start=(d == 0), stop=(d == DC - 1))
                    pt = trans.tile([128, 256], F16, tag=f"pt{tj}", name="pt_t",
                                    bufs=2)
                    nc.scalar.activation(pt[:, 0:w], psS[:, 0:w],
                                         mybir.ActivationFunctionType.Exp)
                    mi = tj - (cp - 4)
                    if mi >= 0:
                        if w == 256:
                            nc.vector.tensor_mul(pt[:], pt[:], msk256[P][mi][:])
                        else:
                            nc.vector.tensor_mul(pt[:, 0:128], pt[:, 0:128],
                                                 msk128[P][mi - 2][:])
                    pts.append((pt, w))
                    for unit in proj_at.get(tj, ()):
                        unit()
                return pts

            def emit_den_pv(P, pts):
                cp = CP[P]
                # den burst first: all exps have retired by now, so these
                # matmuls never make the PE wait on the ACT/DVE chain.
                pden = pden_pool.tile([1, 256], F32, tag="den", name="pden_t")
                for tj in range(cp):
                    pt, w = pts[tj]
                    nc.tensor.matmul(pden[:, 0:w], ones_bf[:], pt[:, 0:w],
                                     start=(tj == 0), stop=(tj == cp - 1))
                den_row = trans.tile([1, 256], F32, tag="denrow", name="den_row")
                nc.vector.tensor_copy(den_row[:], pden[:])
                nc.sync.dma_start(den_dram[ds(256 * P, 256)], den_row[:])
                den_col = trans.tile([128, 2], F32, tag="dencol", name="den_col",
                                     bufs=2)
                nc.sync.dma_start(den_col[:],
                                  den_dram[ds(256 * P, 256)]
                                  .rearrange("(t p) -> p t", p=128))
                recip = trans.tile([128, 2], F32, tag="recip", name="recip",
                                   bufs=2)
                nc.vector.reciprocal(recip[:], den_col[:])
                recip_all[P] = recip

                if phases <= 2:
                    return
                # PV flipped: stationary = P-tile slot columns, moving = VP
                # rows (Wp is host-folded into VP), so the accumulator lands
                # row-major [q, dims] — it IS the final pre-bias output; the
                # per-row 1/den scale applies per-partition and no on-device
                # projection or transpose is needed.
                for slot in range(2):
                    # slot1 (the low tile of the pair) never needs the last
                    # two j-blocks; masks already zero its dead region
                    ntj = cp if slot == 0 else cp - 2
                    po = po_pool.tile([128, 1024], F32, tag=f"poF{slot}",
                                      name=f"poF{slot}_t")
                    for half in range(2):
                        for tj in range(ntj):
                            pt, w = pts[tj]
                            nc.tensor.matmul(po[:, ds(512 * half, 512)],
                                             pt[:, ds(128 * slot, 128)],
                                             v_sb[tj][:, ds(512 * half, 512)],
                                             start=(tj == 0),
                                             stop=(tj == ntj - 1))
                    ob = trans.tile([128, 1024], F16, tag="obF", name="obF_t",
                                    bufs=2)
                    nc.vector.tensor_scalar_mul(
                        ob[:, 0:512], po[:, 0:512],
                        recip_all[P][:, slot:slot + 1])
                    nc.scalar.activation(
                        ob[:, 512:1024], po[:, 512:1024],
                        mybir.ActivationFunctionType.Identity,
                        scale=recip_all[P][:, slot:slot + 1])
                    nc.sync.dma_start(out[ds(128 * (2 * P + slot), 128), :],
                                      ob[:])

            # biggest pair last: its long PV fully covers the den-reciprocal
            # round trip, so the final output scale never waits
            for P in (3, 2, 1, 0):
                pts = emit_scores(P)
                emit_den_pv(P, pts)


def _host_masks(par):
    """Uniform-template masks.

    m256[P, i] multiplies the P tile at tj = cp-4+i (full 256-wide blocks);
    m128[P, i] multiplies the slot0 half at tj = cp-2+i (128-wide blocks).
    parity0 pair P owns (hi, lo) = (cp-1, cp-4); parity1 owns (cp-2, cp-3).
    """
    m256 = np.zeros((4, 2, 128, 256), np.float32)
    m128 = np.zeros((4, 2, 128, 128), np.float32)
    j = np.arange(128)[:, None]
    i = np.arange(128)[None, :]
    tri = (j <= i).astype(np.float32)   # diagonal tile mask
    ones = np.ones((128, 128), np.float32)
    zeros = np.zeros((128, 128), np.float32)
    for P in range(4):
        cp = CP[P]
        if par == 0:
            # hi = cp-1 at slot0, lo = cp-4 at slot1
            m256[P, 0, :, 0:128] = ones   # tj=cp-4 vs hi: below diag
            m256[P, 0, :, 128:256] = tri  # tj=cp-4 == lo: diagonal
            m256[P, 1, :, 0:128] = ones   # tj=cp-3 vs hi: below diag
            m256[P, 1, :, 128:256] = zeros  # tj=cp-3 > lo: dead
            m128[P, 0] = ones             # tj=cp-2 < hi
            m128[P, 1] = tri              # tj=cp-1 == hi: diagonal
        else:
            # hi = cp-2 at slot0, lo = cp-3 at slot1
            m256[P, 0, :, 0:128] = ones   # tj=cp-4 < hi
            m256[P, 0, :, 128:256] = ones  # tj=cp-4 < lo
            m256[P, 1, :, 0:128] = ones   # tj=cp-3 < hi
            m256[P, 1, :, 128:256] = tri  # tj=cp-3 == lo: diagonal
            m128[P, 0] = tri              # tj=cp-2 == hi: diagonal
            m128[P, 1] = zeros            # tj=cp-1 > hi: dead
    return m256.astype(np.float16), m128.astype(np.float16)


def kernel(x, W_attn, b_attn, W_proj, b_proj, _repeat=1, _results_only=False,
           _phases=3):
    x = np.asarray(x, np.float32)
    W_attn = np.asarray(W_attn, np.float32)
    b_attn = np.asarray(b_attn, np.float32)
    W_proj = np.asarray(W_proj, np.float32)
    b_proj = np.asarray(b_proj, np.float32)
    B = x.shape[0]

    nc = _build(_repeat, _phases)

    b_eff = (b_proj.astype(np.float64)
             + b_attn[2 * D:].astype(np.float64) @ W_proj.astype(np.float64)
             ).astype(np.float32)
    # exact pow-2 rescales: q,bq /8 (softmax scale); v /8 and wp *8 (fp16 range)
    def stat_layout(w):
        # [D, D] -> [m, p, c*128+f] with w[c*128+p][m*128+f]
        return np.ascontiguousarray(
            w.reshape(DC, 128, DC, 128).transpose(2, 1, 0, 3).reshape(DC, 128, D))

    wq = stat_layout((W_attn[:, :D] * np.float32(0.125)).astype(np.float16))
    wk = stat_layout(W_attn[:, D:2 * D].astype(np.float16))
    # fold the output projection into the value weights (associativity:
    # (P@V)@Wp = P@(X@(Wv@Wp))) — the device then needs no projection phase
    wv = (W_attn[:, 2 * D:].astype(np.float64)
          @ W_proj.astype(np.float64)).astype(np.float16)
    bqv = (b_attn[:D] * np.float32(0.125)).astype(np.float32)
    masks_by_par = [_host_masks(0), _host_masks(1)]

    in_maps = []
    for c in range(8):
        b, par = c // 2, c % 2
        own = OWN[par]
        xTb = np.ascontiguousarray(x[b].T.astype(np.float16))
        cols = np.concatenate([np.arange(128 * t, 128 * (t + 1)) for t in own])
        xqT = np.ascontiguousarray(xTb[:, cols])
        m256, m128 = masks_by_par[par]
        in_maps.append({"xT": xTb, "xqT": xqT, "wq": wq, "wk": wk, "wv": wv,
                        "bq": bqv, "m256": m256, "m128": m128})

    res = run_bass_kernel_spmd(nc, in_maps, core_ids=list(range(8)))
    if _results_only:
        return res

    out = np.empty((B, T, D), np.float32)
    for c in range(8):
        b, par = c // 2, c % 2
        part = res.results[c]["out"].astype(np.float32)
        for s, t in enumerate(OWN[par]):
            out[b, 128 * t:128 * (t + 1), :] = part[128 * s:128 * (s + 1), :] + b_eff
    return out
